# revision 2
# baseline (speedup 1.0000x reference)
import numpy as np
import jax
import jax.numpy as jnp
from functools import partial
from jax.sharding import Mesh, PartitionSpec as P
from jax.experimental.shard_map import shard_map

# nn_AtomCrossAttDecoder constants (hardcoded per spec)
T, A, S, Q, K = 1024, 24, 512, 32, 128
C, CT, CP, NB, NH, HD = 128, 768, 16, 3, 4, 32
FH = 2 * C
NCORES = 8


def _ln(x, scale=None, eps=1e-5):
    mu = jnp.mean(x, axis=-1, keepdims=True)
    var = jnp.var(x, axis=-1, keepdims=True)
    y = (x - mu) * jax.lax.rsqrt(var + eps)
    return y * scale if scale is not None else y


def _forward_shard(token_act, skip_connection, queries_single_cond, keys_single_cond,
                   pair_cond, w_proj, pair_ln_scale, w_pair,
                   qln_cond_scale, qln_wscale, qln_bscale, qln_wbias,
                   kln_cond_scale, kln_wscale, kln_bscale, kln_wbias,
                   wq, wk, wv, wgate, bgate, wout,
                   tln_cond_scale, tln_wscale, tln_bscale, tln_wbias,
                   wtrans_in, wtrans_out, wtgate, btgate,
                   final_ln_scale, w_pos,
                   a2q_idx, a2q_mask, q2k_idx, q2k_mask, q2a_idx, q2a_mask,
                   queries_mask, keys_mask):
    # Everything with a leading S axis arrives sharded (Sl = S/8 subsets);
    # q2a_idx/q2a_mask arrive sharded over T. Weights replicated.
    Sl = skip_connection.shape[0]
    qm = queries_mask[..., None].astype(jnp.float32)

    tok = token_act @ w_proj                              # (T, C) replicated compute
    # flat (T*A) gather == tok row gather via idx // A
    x = tok[a2q_idx // A] * a2q_mask[..., None].astype(jnp.float32)
    x = (x + skip_connection) * qm

    pl = _ln(pair_cond, pair_ln_scale) @ w_pair
    pl = pl.reshape(Sl, Q, K, NB, NH).transpose(3, 0, 4, 1, 2)  # (NB, Sl, NH, Q, K)

    cq_n = _ln(queries_single_cond)                       # (Sl, Q, C) unscaled
    ck_n = _ln(keys_single_cond)                          # (Sl, K, C)

    for b in range(NB):
        # cross-subset gather: all-gather local x, then index the flat grid
        xg = jax.lax.all_gather(x, 'i', tiled=True)       # (S, Q, C)
        keys_act = xg.reshape(S * Q, C)[q2k_idx] * q2k_mask[..., None].astype(jnp.float32)
        cq = cq_n * qln_cond_scale[b]
        ck = ck_n * kln_cond_scale[b]
        qn = jax.nn.sigmoid(cq @ qln_wscale[b] + qln_bscale[b]) * _ln(x) + cq @ qln_wbias[b]
        kn = jax.nn.sigmoid(ck @ kln_wscale[b] + kln_bscale[b]) * _ln(keys_act) + ck @ kln_wbias[b]
        q = (qn @ wq[b]).reshape(Sl, Q, NH, HD)
        k = (kn @ wk[b]).reshape(Sl, K, NH, HD)
        v = (kn @ wv[b]).reshape(Sl, K, NH, HD)
        logits = jnp.einsum('sqhd,skhd->shqk', q, k) * (HD ** -0.5) + pl[b]
        logits = jnp.where(keys_mask[:, None, None, :], logits, jnp.float32(-1e9))
        attn = jax.nn.softmax(logits, axis=-1)
        o = jnp.einsum('shqk,skhd->sqhd', attn, v).reshape(Sl, Q, NH * HD)
        gate = jax.nn.sigmoid(cq @ wgate[b] + bgate[b])
        x = x + gate * (o @ wout[b])
        ct = cq_n * tln_cond_scale[b]
        xt = jax.nn.sigmoid(ct @ tln_wscale[b] + tln_bscale[b]) * _ln(x) + ct @ tln_wbias[b]
        h = xt @ wtrans_in[b]
        a, g = jnp.split(h, 2, axis=-1)
        out_t = (jax.nn.swish(a) * g) @ wtrans_out[b]
        x = x + jax.nn.sigmoid(ct @ wtgate[b] + btgate[b]) * out_t

    x = _ln(x * qm, final_ln_scale)
    pos = x @ w_pos                                       # (Sl, Q, 3)
    posg = jax.lax.all_gather(pos, 'i', tiled=True).reshape(S * Q, 3)
    out = posg[q2a_idx] * q2a_mask[..., None].astype(jnp.float32)  # (T/8, A, 3)
    return out


_jitted = None


def _get_fn():
    global _jitted
    if _jitted is not None:
        return _jitted
    devs = jax.devices()[:NCORES]
    mesh = Mesh(np.array(devs), ('i',))
    shard = P('i')
    repl = P()
    in_specs = (
        repl,   # token_act
        shard,  # skip_connection
        shard,  # queries_single_cond
        shard,  # keys_single_cond
        shard,  # pair_cond
        repl, repl, repl,                    # w_proj, pair_ln_scale, w_pair
        repl, repl, repl, repl,              # qln_*
        repl, repl, repl, repl,              # kln_*
        repl, repl, repl, repl, repl, repl,  # wq wk wv wgate bgate wout
        repl, repl, repl, repl,              # tln_*
        repl, repl, repl, repl,              # wtrans_in wtrans_out wtgate btgate
        repl, repl,                          # final_ln_scale, w_pos
        shard, shard,                        # a2q_idx, a2q_mask   (S axis)
        shard, shard,                        # q2k_idx, q2k_mask   (S axis)
        shard, shard,                        # q2a_idx, q2a_mask   (T axis)
        shard, shard,                        # queries_mask, keys_mask (S axis)
    )
    fn = shard_map(_forward_shard, mesh=mesh, in_specs=in_specs, out_specs=P('i'))
    _jitted = jax.jit(fn)
    return _jitted


_ORDER = ['token_act', 'skip_connection', 'queries_single_cond', 'keys_single_cond',
          'pair_cond', 'w_proj', 'pair_ln_scale', 'w_pair',
          'qln_cond_scale', 'qln_wscale', 'qln_bscale', 'qln_wbias',
          'kln_cond_scale', 'kln_wscale', 'kln_bscale', 'kln_wbias',
          'wq', 'wk', 'wv', 'wgate', 'bgate', 'wout',
          'tln_cond_scale', 'tln_wscale', 'tln_bscale', 'tln_wbias',
          'wtrans_in', 'wtrans_out', 'wtgate', 'btgate',
          'final_ln_scale', 'w_pos',
          'a2q_idx', 'a2q_mask', 'q2k_idx', 'q2k_mask', 'q2a_idx', 'q2a_mask',
          'queries_mask', 'keys_mask']


def kernel(**inputs) -> np.ndarray:
    fn = _get_fn()
    args = [jnp.asarray(inputs[k]) for k in _ORDER]
    out = fn(*args)
    return np.asarray(jax.block_until_ready(out))


# revision 3
# speedup vs baseline: 71.8281x; 71.8281x over previous
import numpy as np
import jax
import jax.numpy as jnp
from functools import partial
from jax.sharding import Mesh, PartitionSpec as P
from jax.experimental.shard_map import shard_map

# nn_AtomCrossAttDecoder constants (hardcoded per spec)
T, A, S, Q, K = 1024, 24, 512, 32, 128
C, CT, CP, NB, NH, HD = 128, 768, 16, 3, 4, 32
FH = 2 * C
NCORES = 8


def _ln(x, scale=None, eps=1e-5):
    mu = jnp.mean(x, axis=-1, keepdims=True)
    var = jnp.var(x, axis=-1, keepdims=True)
    y = (x - mu) * jax.lax.rsqrt(var + eps)
    return y * scale if scale is not None else y


def _forward_shard(token_act, skip_connection, queries_single_cond, keys_single_cond,
                   pair_cond, w_proj, pair_ln_scale, w_pair,
                   qln_cond_scale, qln_wscale, qln_bscale, qln_wbias,
                   kln_cond_scale, kln_wscale, kln_bscale, kln_wbias,
                   wq, wk, wv, wgate, bgate, wout,
                   tln_cond_scale, tln_wscale, tln_bscale, tln_wbias,
                   wtrans_in, wtrans_out, wtgate, btgate,
                   final_ln_scale, w_pos,
                   a2q_idx, a2q_mask, q2k_idx, q2k_mask, q2a_idx, q2a_mask,
                   queries_mask, keys_mask):
    # Everything with a leading S axis arrives sharded (Sl = S/8 subsets);
    # q2a_idx/q2a_mask arrive sharded over T. Weights replicated.
    with jax.default_matmul_precision('bfloat16'):
        return _forward_body(token_act, skip_connection, queries_single_cond,
                             keys_single_cond, pair_cond, w_proj, pair_ln_scale, w_pair,
                             qln_cond_scale, qln_wscale, qln_bscale, qln_wbias,
                             kln_cond_scale, kln_wscale, kln_bscale, kln_wbias,
                             wq, wk, wv, wgate, bgate, wout,
                             tln_cond_scale, tln_wscale, tln_bscale, tln_wbias,
                             wtrans_in, wtrans_out, wtgate, btgate,
                             final_ln_scale, w_pos,
                             a2q_idx, a2q_mask, q2k_idx, q2k_mask, q2a_idx, q2a_mask,
                             queries_mask, keys_mask)


def _forward_body(token_act, skip_connection, queries_single_cond, keys_single_cond,
                  pair_cond, w_proj, pair_ln_scale, w_pair,
                  qln_cond_scale, qln_wscale, qln_bscale, qln_wbias,
                  kln_cond_scale, kln_wscale, kln_bscale, kln_wbias,
                  wq, wk, wv, wgate, bgate, wout,
                  tln_cond_scale, tln_wscale, tln_bscale, tln_wbias,
                  wtrans_in, wtrans_out, wtgate, btgate,
                  final_ln_scale, w_pos,
                  a2q_idx, a2q_mask, q2k_idx, q2k_mask, q2a_idx, q2a_mask,
                  queries_mask, keys_mask):
    Sl = skip_connection.shape[0]
    qm = queries_mask[..., None].astype(jnp.float32)

    tok = token_act @ w_proj                              # (T, C) replicated compute
    # flat (T*A) gather == tok row gather via idx // A
    x = tok[a2q_idx // A] * a2q_mask[..., None].astype(jnp.float32)
    x = (x + skip_connection) * qm

    pl = _ln(pair_cond, pair_ln_scale) @ w_pair
    pl = pl.reshape(Sl, Q, K, NB, NH).transpose(3, 0, 4, 1, 2)  # (NB, Sl, NH, Q, K)

    cq_n = _ln(queries_single_cond)                       # (Sl, Q, C) unscaled
    ck_n = _ln(keys_single_cond)                          # (Sl, K, C)

    for b in range(NB):
        # cross-subset gather: all-gather local x, then index the flat grid
        xg = jax.lax.all_gather(x, 'i', tiled=True)       # (S, Q, C)
        keys_act = xg.reshape(S * Q, C)[q2k_idx] * q2k_mask[..., None].astype(jnp.float32)
        cq = cq_n * qln_cond_scale[b]
        ck = ck_n * kln_cond_scale[b]
        qn = jax.nn.sigmoid(cq @ qln_wscale[b] + qln_bscale[b]) * _ln(x) + cq @ qln_wbias[b]
        kn = jax.nn.sigmoid(ck @ kln_wscale[b] + kln_bscale[b]) * _ln(keys_act) + ck @ kln_wbias[b]
        q = (qn @ wq[b]).reshape(Sl, Q, NH, HD)
        k = (kn @ wk[b]).reshape(Sl, K, NH, HD)
        v = (kn @ wv[b]).reshape(Sl, K, NH, HD)
        logits = jnp.einsum('sqhd,skhd->shqk', q, k) * (HD ** -0.5) + pl[b]
        logits = jnp.where(keys_mask[:, None, None, :], logits, jnp.float32(-1e9))
        attn = jax.nn.softmax(logits, axis=-1)
        o = jnp.einsum('shqk,skhd->sqhd', attn, v).reshape(Sl, Q, NH * HD)
        gate = jax.nn.sigmoid(cq @ wgate[b] + bgate[b])
        x = x + gate * (o @ wout[b])
        ct = cq_n * tln_cond_scale[b]
        xt = jax.nn.sigmoid(ct @ tln_wscale[b] + tln_bscale[b]) * _ln(x) + ct @ tln_wbias[b]
        h = xt @ wtrans_in[b]
        a, g = jnp.split(h, 2, axis=-1)
        out_t = (jax.nn.swish(a) * g) @ wtrans_out[b]
        x = x + jax.nn.sigmoid(ct @ wtgate[b] + btgate[b]) * out_t

    x = _ln(x * qm, final_ln_scale)
    pos = x @ w_pos                                       # (Sl, Q, 3)
    posg = jax.lax.all_gather(pos, 'i', tiled=True).reshape(S * Q, 3)
    out = posg[q2a_idx] * q2a_mask[..., None].astype(jnp.float32)  # (T/8, A, 3)
    return out


_jitted = None


def _get_fn():
    global _jitted
    if _jitted is not None:
        return _jitted
    devs = jax.devices()[:NCORES]
    mesh = Mesh(np.array(devs), ('i',))
    shard = P('i')
    repl = P()
    in_specs = (
        repl,   # token_act
        shard,  # skip_connection
        shard,  # queries_single_cond
        shard,  # keys_single_cond
        shard,  # pair_cond
        repl, repl, repl,                    # w_proj, pair_ln_scale, w_pair
        repl, repl, repl, repl,              # qln_*
        repl, repl, repl, repl,              # kln_*
        repl, repl, repl, repl, repl, repl,  # wq wk wv wgate bgate wout
        repl, repl, repl, repl,              # tln_*
        repl, repl, repl, repl,              # wtrans_in wtrans_out wtgate btgate
        repl, repl,                          # final_ln_scale, w_pos
        shard, shard,                        # a2q_idx, a2q_mask   (S axis)
        shard, shard,                        # q2k_idx, q2k_mask   (S axis)
        shard, shard,                        # q2a_idx, q2a_mask   (T axis)
        shard, shard,                        # queries_mask, keys_mask (S axis)
    )
    fn = shard_map(_forward_shard, mesh=mesh, in_specs=in_specs, out_specs=P('i'))
    _jitted = jax.jit(fn)
    return _jitted


_ORDER = ['token_act', 'skip_connection', 'queries_single_cond', 'keys_single_cond',
          'pair_cond', 'w_proj', 'pair_ln_scale', 'w_pair',
          'qln_cond_scale', 'qln_wscale', 'qln_bscale', 'qln_wbias',
          'kln_cond_scale', 'kln_wscale', 'kln_bscale', 'kln_wbias',
          'wq', 'wk', 'wv', 'wgate', 'bgate', 'wout',
          'tln_cond_scale', 'tln_wscale', 'tln_bscale', 'tln_wbias',
          'wtrans_in', 'wtrans_out', 'wtgate', 'btgate',
          'final_ln_scale', 'w_pos',
          'a2q_idx', 'a2q_mask', 'q2k_idx', 'q2k_mask', 'q2a_idx', 'q2a_mask',
          'queries_mask', 'keys_mask']


def kernel(**inputs) -> np.ndarray:
    fn = _get_fn()
    args = [jnp.asarray(inputs[k]) for k in _ORDER]
    out = fn(*args)
    return np.asarray(jax.block_until_ready(out))


# revision 4
# speedup vs baseline: 109.0871x; 1.5187x over previous
import numpy as np
import jax
import jax.numpy as jnp
from functools import partial
from jax.sharding import Mesh, PartitionSpec as P
from jax.experimental.shard_map import shard_map

# nn_AtomCrossAttDecoder constants (hardcoded per spec)
T, A, S, Q, K = 1024, 24, 512, 32, 128
C, CT, CP, NB, NH, HD = 128, 768, 16, 3, 4, 32
FH = 2 * C
NCORES = 8


def _ln(x, scale=None, eps=1e-5):
    mu = jnp.mean(x, axis=-1, keepdims=True)
    var = jnp.var(x, axis=-1, keepdims=True)
    y = (x - mu) * jax.lax.rsqrt(var + eps)
    return y * scale if scale is not None else y


def _forward_shard(token_act, skip_connection, queries_single_cond, keys_single_cond,
                   pair_cond, w_proj, pair_ln_scale, w_pair,
                   qln_cond_scale, qln_wscale, qln_bscale, qln_wbias,
                   kln_cond_scale, kln_wscale, kln_bscale, kln_wbias,
                   wq, wk, wv, wgate, bgate, wout,
                   tln_cond_scale, tln_wscale, tln_bscale, tln_wbias,
                   wtrans_in, wtrans_out, wtgate, btgate,
                   final_ln_scale, w_pos,
                   a2q_idx, a2q_mask, q2k_idx, q2k_mask, q2a_idx, q2a_mask,
                   queries_mask, keys_mask):
    # Everything with a leading S axis arrives sharded (Sl = S/8 subsets);
    # q2a_idx/q2a_mask arrive sharded over T. Weights replicated.
    with jax.default_matmul_precision('bfloat16'):
        return _forward_body(token_act, skip_connection, queries_single_cond,
                             keys_single_cond, pair_cond, w_proj, pair_ln_scale, w_pair,
                             qln_cond_scale, qln_wscale, qln_bscale, qln_wbias,
                             kln_cond_scale, kln_wscale, kln_bscale, kln_wbias,
                             wq, wk, wv, wgate, bgate, wout,
                             tln_cond_scale, tln_wscale, tln_bscale, tln_wbias,
                             wtrans_in, wtrans_out, wtgate, btgate,
                             final_ln_scale, w_pos,
                             a2q_idx, a2q_mask, q2k_idx, q2k_mask, q2a_idx, q2a_mask,
                             queries_mask, keys_mask)


def _forward_body(token_act, skip_connection, queries_single_cond, keys_single_cond,
                  pair_cond, w_proj, pair_ln_scale, w_pair,
                  qln_cond_scale, qln_wscale, qln_bscale, qln_wbias,
                  kln_cond_scale, kln_wscale, kln_bscale, kln_wbias,
                  wq, wk, wv, wgate, bgate, wout,
                  tln_cond_scale, tln_wscale, tln_bscale, tln_wbias,
                  wtrans_in, wtrans_out, wtgate, btgate,
                  final_ln_scale, w_pos,
                  a2q_idx, a2q_mask, q2k_idx, q2k_mask, q2a_idx, q2a_mask,
                  queries_mask, keys_mask):
    Sl = skip_connection.shape[0]
    qm = queries_mask[..., None].astype(jnp.float32)

    tok = token_act @ w_proj                              # (T, C) replicated compute
    # flat (T*A) gather == tok row gather via idx // A
    x = tok[a2q_idx // A] * a2q_mask[..., None].astype(jnp.float32)
    x = (x + skip_connection) * qm

    pl = _ln(pair_cond, pair_ln_scale) @ w_pair
    pl = pl.reshape(Sl, Q, K, NB, NH).transpose(3, 0, 4, 1, 2)  # (NB, Sl, NH, Q, K)

    cq_n = _ln(queries_single_cond)                       # (Sl, Q, C) unscaled
    ck_n = _ln(keys_single_cond)                          # (Sl, K, C)

    for b in range(NB):
        # cross-subset gather: all-gather local x (bf16 to halve collective
        # bytes; keys path tolerance is ample), then index the flat grid
        xg = jax.lax.all_gather(x.astype(jnp.bfloat16), 'i', tiled=True)  # (S, Q, C)
        keys_act = (xg.reshape(S * Q, C)[q2k_idx] * q2k_mask[..., None].astype(jnp.bfloat16)
                    ).astype(jnp.float32)
        cq = cq_n * qln_cond_scale[b]
        ck = ck_n * kln_cond_scale[b]
        qn = jax.nn.sigmoid(cq @ qln_wscale[b] + qln_bscale[b]) * _ln(x) + cq @ qln_wbias[b]
        kn = jax.nn.sigmoid(ck @ kln_wscale[b] + kln_bscale[b]) * _ln(keys_act) + ck @ kln_wbias[b]
        q = (qn @ wq[b]).reshape(Sl, Q, NH, HD)
        k = (kn @ wk[b]).reshape(Sl, K, NH, HD)
        v = (kn @ wv[b]).reshape(Sl, K, NH, HD)
        logits = jnp.einsum('sqhd,skhd->shqk', q, k) * (HD ** -0.5) + pl[b]
        logits = jnp.where(keys_mask[:, None, None, :], logits, jnp.float32(-1e9))
        attn = jax.nn.softmax(logits, axis=-1)
        o = jnp.einsum('shqk,skhd->sqhd', attn, v).reshape(Sl, Q, NH * HD)
        gate = jax.nn.sigmoid(cq @ wgate[b] + bgate[b])
        x = x + gate * (o @ wout[b])
        ct = cq_n * tln_cond_scale[b]
        xt = jax.nn.sigmoid(ct @ tln_wscale[b] + tln_bscale[b]) * _ln(x) + ct @ tln_wbias[b]
        h = xt @ wtrans_in[b]
        a, g = jnp.split(h, 2, axis=-1)
        out_t = (jax.nn.swish(a) * g) @ wtrans_out[b]
        x = x + jax.nn.sigmoid(ct @ wtgate[b] + btgate[b]) * out_t

    x = _ln(x * qm, final_ln_scale)
    pos = x @ w_pos                                       # (Sl, Q, 3)
    posg = jax.lax.all_gather(pos, 'i', tiled=True).reshape(S * Q, 3)
    out = posg[q2a_idx] * q2a_mask[..., None].astype(jnp.float32)  # (T/8, A, 3)
    return out


_jitted = None


def _get_fn():
    global _jitted
    if _jitted is not None:
        return _jitted
    devs = jax.devices()[:NCORES]
    mesh = Mesh(np.array(devs), ('i',))
    shard = P('i')
    repl = P()
    in_specs = (
        repl,   # token_act
        shard,  # skip_connection
        shard,  # queries_single_cond
        shard,  # keys_single_cond
        shard,  # pair_cond
        repl, repl, repl,                    # w_proj, pair_ln_scale, w_pair
        repl, repl, repl, repl,              # qln_*
        repl, repl, repl, repl,              # kln_*
        repl, repl, repl, repl, repl, repl,  # wq wk wv wgate bgate wout
        repl, repl, repl, repl,              # tln_*
        repl, repl, repl, repl,              # wtrans_in wtrans_out wtgate btgate
        repl, repl,                          # final_ln_scale, w_pos
        shard, shard,                        # a2q_idx, a2q_mask   (S axis)
        shard, shard,                        # q2k_idx, q2k_mask   (S axis)
        shard, shard,                        # q2a_idx, q2a_mask   (T axis)
        shard, shard,                        # queries_mask, keys_mask (S axis)
    )
    fn = shard_map(_forward_shard, mesh=mesh, in_specs=in_specs, out_specs=P('i'))
    _jitted = jax.jit(fn)
    return _jitted


_ORDER = ['token_act', 'skip_connection', 'queries_single_cond', 'keys_single_cond',
          'pair_cond', 'w_proj', 'pair_ln_scale', 'w_pair',
          'qln_cond_scale', 'qln_wscale', 'qln_bscale', 'qln_wbias',
          'kln_cond_scale', 'kln_wscale', 'kln_bscale', 'kln_wbias',
          'wq', 'wk', 'wv', 'wgate', 'bgate', 'wout',
          'tln_cond_scale', 'tln_wscale', 'tln_bscale', 'tln_wbias',
          'wtrans_in', 'wtrans_out', 'wtgate', 'btgate',
          'final_ln_scale', 'w_pos',
          'a2q_idx', 'a2q_mask', 'q2k_idx', 'q2k_mask', 'q2a_idx', 'q2a_mask',
          'queries_mask', 'keys_mask']


def kernel(**inputs) -> np.ndarray:
    fn = _get_fn()
    args = [jnp.asarray(inputs[k]) for k in _ORDER]
    out = fn(*args)
    return np.asarray(jax.block_until_ready(out))
